# revision 7
# baseline (speedup 1.0000x reference)
"""Trainium2 Bass kernel for nn_LinearAttention (RoPE(Q) @ RoPE(Q)^T @ V).

Algebra: no softmax, so out = (QR @ QR^T) @ V == QR @ (QR^T @ V) with a
[d,d] (64x64) intermediate per head. Sharding: 16 heads / 8 cores = 2
heads per core, no cross-core traffic. The two heads ride the two
64-lane halves of the 128x128 PE array.

Layout: t = p*16 + r*8 + c (p = SBUF partition, r = range 0/1, c =
chunk-in-range); the host packs/unpacks with this permutation.

v2 changes vs the first working kernel (31.1us):
  * Tables shipped COMPACT ([r,c,k] cos/sin, no per-head repeat: 288KB
    instead of 544KB) and broadcast over h with stride-0 APs in the
    RoPE muls -- cuts the input stream by 20%.
  * RoPE is DVE-only. GpSimd tensor ops are gone: every DVE
    tensor_tensor needs the shared SBUF read port that GpSimd locks
    for its whole (4x slower) instruction, so DVE+GpSimd elementwise
    work serializes instead of overlapping (measured 3x slowdown).
  * 12 RoPE ops of [128,512], all reads/writes contiguous (the chunk
    strides moved into the matmul lhsT APs, which tolerate them).
  * Transposes batch 4 chunks into one PSUM bank -> 4 big [128,512]
    evacuation casts on ACT instead of 16 small copies split DVE/ACT.
  * Phase-3 uses 4 distinct PSUM banks so its matmuls stream
    back-to-back; evac casts alternate DVE/ACT; output DMAs alternate
    the two HWDGE rings.
  * Input DMA instructions are hoisted into the engine preamble block
    (before the initial all-engine barrier) -- they have no waits, and
    issuing them ~0.8us earlier starts the HBM stream during the
    barrier/branch overhead.
  * PE warm-up spam sized to bridge from the preamble to the first
    real matmul, plus a short mid-kernel bridge while DVE finishes
    RoPE-B, so HAM stays at K=8/8 for the phase-3 matmuls.
"""

from contextlib import ExitStack

import numpy as np

import concourse.bass as bass
import concourse.mybir as mybir
import concourse.tile as tile
from concourse.bass_utils import run_bass_kernel_spmd
from concourse.vector_clock import ScopedClock

H, T, D = 16, 2048, 64
N_CORES = 8
HPC = H // N_CORES  # heads per core
P = 128
NT = T // P  # 16 t-chunks per head
HD = D // 2
NTAB = 2 * 2 * 8 * HD + P  # cosA|sinA|cosB|sinB ([c,k] each) | idt
F32 = mybir.dt.float32
BF16 = mybir.dt.bfloat16
N_WARM = 26  # leading dep-free transposes: preamble -> first real MM
N_WARM_MID = 5  # bridge the PE gap while DVE finishes RoPE-B


def _rope_tables():
    inv_freq = 1.0 / (10000.0 ** (np.arange(0, D, 2, dtype=np.float32) / D))
    t = np.arange(T, dtype=np.float32)
    freqs = np.outer(t, inv_freq).astype(np.float32)  # [T, D/2]
    return np.cos(freqs).astype(np.float32), np.sin(freqs).astype(np.float32)


class _SlimTileContext(tile.TileContext):
    """TileContext whose kernel tail uses per-engine drains + a
    sequencer-level (sem-only) barrier instead of the full EVSEM
    butterfly (~8us)."""

    def _drain_and_barrier(self, tick_clock, wait_clock):
        nc = self.nc
        drain_inst = nc.sync.drain()
        wait_clock.add_sem_waits(
            drain_inst.ins, ScopedClock({None: tick_clock.global_clock})
        )
        for eng in nc.engines.values():
            if eng.engine != mybir.EngineType.SP:
                eng.drain(fusable=False)
        nc.all_engine_barrier(sem_only=True)
        popped = nc._tile_sem_poison_stack.pop()
        assert popped is self._sem_poison
        nc.clear_and_free_semaphores(list(self.sems.allocated().values()))
        nc.all_engine_barrier(sem_only=True)


def _build_nc():
    nc = bass.Bass()
    TAB = nc.declare_dram_parameter("TAB", [P, NTAB], BF16, isOutput=False)
    # q: [p, (x h c k)] per range (x = rotate-half half, h = head)
    QA = nc.declare_dram_parameter("QA", [P, 1024], BF16, isOutput=False)
    QB = nc.declare_dram_parameter("QB", [P, 1024], BF16, isOutput=False)
    # v: [p, (c h d)] per range
    VA = nc.declare_dram_parameter("VA", [P, 1024], BF16, isOutput=False)
    VB = nc.declare_dram_parameter("VB", [P, 1024], BF16, isOutput=False)
    OUT = nc.declare_dram_parameter("OUT", [P, T], BF16, isOutput=True)

    with _SlimTileContext(nc) as tc, ExitStack() as ctx:
        singles = ctx.enter_context(tc.tile_pool(name="singles", bufs=1))
        ps_s = ctx.enter_context(tc.tile_pool(name="ps_s", bufs=1, space="PSUM"))
        ps_tp = ctx.enter_context(tc.tile_pool(name="ps_tp", bufs=2, space="PSUM"))
        ps_o = ctx.enter_context(tc.tile_pool(name="ps_o", bufs=4, space="PSUM"))
        ps_w = ctx.enter_context(tc.tile_pool(name="ps_w", bufs=1, space="PSUM"))

        tab_sb = singles.tile([P, NTAB], BF16)
        # q layout: [p, r, x, h, c, k]
        q_sb = singles.tile([P, 2, 2, HPC, 8, HD], BF16)
        v_sb = singles.tile([P, NT, P], BF16)
        # chunk-major so each chunk's (h,x,k) is a contiguous 128-elem
        # lhsT slice (matmul stationary APs allow only one free dim)
        qr_sb = singles.tile([P, NT, HPC, 2, HD], BF16)
        tm = singles.tile([P, 4, HPC, 8, HD], BF16)
        qrt_sb = singles.tile([P, NT * P], BF16)
        s2d = singles.tile([P, P], BF16)
        outT_sb = singles.tile([P, T], BF16)
        spam_src = singles.tile([P, P], F32)

        # s2d off-diagonal zeros + spam seed: GpSimd is otherwise unused
        # and runs these during the DMA wait, before any DVE op exists
        # to contend with on the shared SBUF port.
        nc.gpsimd.memset(spam_src[:, 0:2], 0.0)
        nc.gpsimd.memset(s2d[0:D, D:P], 0.0)
        nc.gpsimd.memset(s2d[D:P, 0:D], 0.0)

        # Input DMAs: two HWDGE rings. sync: QA,QB; scalar: TAB,VA,VB.
        # (_hoist_input_dmas moves these into the preamble block.)
        nc.sync.dma_start(
            out=q_sb[:, 0],
            in_=QA[:].rearrange("p (x h c k) -> p x h c k", x=2, h=HPC, c=8),
        )
        nc.scalar.dma_start(out=tab_sb, in_=TAB[:])
        nc.sync.dma_start(
            out=q_sb[:, 1],
            in_=QB[:].rearrange("p (x h c k) -> p x h c k", x=2, h=HPC, c=8),
        )
        nc.scalar.dma_start(
            out=v_sb[:, 0:8], in_=VA[:].rearrange("p (c f) -> p c f", c=8)
        )
        nc.scalar.dma_start(
            out=v_sb[:, 8:16], in_=VB[:].rearrange("p (c f) -> p c f", c=8)
        )

        # Garbage-input PE warm-up: dep-free, keeps HAM busy from the
        # preamble until the first real matmul.
        for _ in range(N_WARM):
            warm = ps_w.tile([P, P], F32, tag="w")
            nc.tensor.transpose(warm, spam_src, spam_src)

        idt = tab_sb[:, 4 * 256 :]  # [P, 128] identity

        def rope(r):
            # 6 contiguous [128,512] DVE ops; cos/sin broadcast over h.
            cosB = (
                tab_sb[:, r * 512 : r * 512 + 256]
                .rearrange("p (c k) -> p c k", c=8)
                .unsqueeze(1)
                .to_broadcast([P, HPC, 8, HD])
            )
            sinB = (
                tab_sb[:, r * 512 + 256 : r * 512 + 512]
                .rearrange("p (c k) -> p c k", c=8)
                .unsqueeze(1)
                .to_broadcast([P, HPC, 8, HD])
            )
            qlo = q_sb[:, r, 0]
            qhi = q_sb[:, r, 1]
            cs = slice(r * 8, r * 8 + 8)
            # combine dests scatter into the chunk-major qr tile using
            # the same (h, c, k) iteration order as the contiguous srcs
            qr_lo = qr_sb[:, cs, :, 0, :].rearrange("p c h k -> p h c k")
            qr_hi = qr_sb[:, cs, :, 1, :].rearrange("p c h k -> p h c k")
            nc.vector.tensor_mul(tm[:, 0], qlo, cosB)
            nc.vector.tensor_mul(tm[:, 1], qhi, sinB)
            nc.vector.tensor_sub(qr_lo, tm[:, 0], tm[:, 1])
            nc.vector.tensor_mul(tm[:, 2], qhi, cosB)
            nc.vector.tensor_mul(tm[:, 3], qlo, sinB)
            nc.vector.tensor_add(qr_hi, tm[:, 2], tm[:, 3])

        s2_ps = ps_s.tile([P, P], F32)

        def phase2(r):
            # per chunk: one LDW (shared) + accum MM + transpose MM;
            # transposes batch 4 chunks per PSUM bank, evacuated by ACT
            # as one [128,512] cast each.
            for ci in range(8):
                c = r * 8 + ci
                if c % 4 == 0:
                    tp = ps_tp.tile([P, 512], F32, tag="tp")
                    phase2.tp = tp
                tp = phase2.tp
                qr_c = qr_sb[:, c].rearrange("p h x k -> p (h x k)")
                nc.tensor.matmul(
                    s2_ps, lhsT=qr_c, rhs=v_sb[:, c],
                    start=(c == 0), stop=(c == NT - 1),
                )
                j = c % 4
                nc.tensor.matmul(
                    tp[:, j * P : (j + 1) * P], lhsT=qr_c, rhs=idt,
                    start=True, stop=True,
                )
                if c % 4 == 3:
                    g = c // 4
                    nc.scalar.copy(out=qrt_sb[:, g * 512 : (g + 1) * 512], in_=tp)

        rope(0)
        phase2(0)
        rope(1)
        # Bridge the PE idle window while DVE finishes RoPE-B.
        for _ in range(N_WARM_MID):
            warm = ps_w.tile([P, P], F32, tag="w")
            nc.tensor.transpose(warm, spam_src, spam_src)
        phase2(1)

        # Diagonal S_h blocks -> block-diagonal phase-3 operand.
        nc.vector.tensor_copy(out=s2d[0:D, 0:D], in_=s2_ps[0:D, 0:D])
        nc.vector.tensor_copy(out=s2d[D:P, D:P], in_=s2_ps[D:P, D:P])

        # outT blocks: blockdiag(S)^T @ QRT serves both heads at once.
        # 4 distinct PSUM banks; evac casts alternate DVE/ACT; output
        # DMAs alternate the two rings.
        for i in range(4):
            o_ps = ps_o.tile([P, 512], F32, tag="o")
            blk = slice(i * 512, (i + 1) * 512)
            nc.tensor.matmul(
                o_ps, lhsT=s2d, rhs=qrt_sb[:, blk], start=True, stop=True
            )
            if i % 2 == 0:
                nc.vector.tensor_copy(out=outT_sb[:, blk], in_=o_ps)
                nc.sync.dma_start(out=OUT[:, blk], in_=outT_sb[:, blk])
            else:
                nc.scalar.copy(out=outT_sb[:, blk], in_=o_ps)
                nc.scalar.dma_start(out=OUT[:, blk], in_=outT_sb[:, blk])

    _split_multi_waits(nc)
    _hoist_input_dmas(nc)
    return nc


def _split_multi_waits(nc):
    """This compiler build rejects instructions carrying more than one
    sync-wait command: split extras into single-wait NoOps placed
    immediately before on the same engine."""
    n = 0
    for f in nc.m.functions:
        for blk in f.blocks:
            new_insts = []
            for inst in blk.instructions:
                si = inst.sync_info
                waits = list(si.on_wait) if si else []
                if len(waits) > 1:
                    for w in waits[:-1]:
                        nop = mybir.InstNoOp(name=f"W-split-{n}", ins=[], outs=[])
                        n += 1
                        nop.engine = inst.engine
                        nop.sync_info = mybir.SyncInfo(on_wait=[w], on_update=[])
                        new_insts.append(nop)
                    inst.sync_info = mybir.SyncInfo(
                        on_wait=[waits[-1]], on_update=list(si.on_update)
                    )
                new_insts.append(inst)
            blk.instructions = new_insts


def _hoist_input_dmas(nc):
    """Move the wait-free input DMA issues from the main block into the
    preamble block, before each issuing engine's drain+barrier, so the
    HBM stream starts ~0.8us earlier (during barrier/branch overhead)."""
    f = nc.m.functions[0]
    if len(f.blocks) < 2:
        return
    pre, main = f.blocks[0], f.blocks[1]
    hoist = []
    for inst in list(main.instructions):
        if isinstance(inst, mybir.InstDMACopy):
            si = inst.sync_info
            if si is not None and len(si.on_wait) > 0:
                continue
            # input loads only: DRAM source (ins reference a DRAM tensor)
            srcs = [x.memref for x in inst.ins] if inst.ins else []
            if any(n.startswith(("QA", "QB", "VA", "VB", "TAB")) for n in srcs):
                hoist.append(inst)
    if not hoist:
        return
    for inst in hoist:
        main.instructions.remove(inst)
    # insert before the issuing engine's first InstDrain in the preamble,
    # preserving issue order within each engine
    for inst in hoist:
        idx = next(
            (
                i
                for i, pi in enumerate(pre.instructions)
                if isinstance(pi, mybir.InstDrain) and pi.engine == inst.engine
            ),
            None,
        )
        if idx is None:
            main.instructions.insert(0, inst)
        else:
            pre.instructions.insert(idx, inst)


_NC_CACHE = None


def _get_nc():
    global _NC_CACHE
    if _NC_CACHE is None:
        _NC_CACHE = _build_nc()
    return _NC_CACHE


def _pack_inputs(Qs, Vs, cos32, sin32, idt):
    import ml_dtypes

    bf16 = ml_dtypes.bfloat16

    # [T, X] -> [P, NT, X] with t = p*NT + u, u = r*8 + c
    def r(x):
        return x.reshape(P, NT, -1)

    # compact tables: tab[p, r, c, k] = cos((p*16 + r*8 + c) * w_k)
    ce = r(cos32).reshape(P, 2, 8, HD)  # [p, r, c, k]
    se = r(sin32).reshape(P, 2, 8, HD)
    tab = np.concatenate(
        [
            ce[:, 0].reshape(P, -1),
            se[:, 0].reshape(P, -1),
            ce[:, 1].reshape(P, -1),
            se[:, 1].reshape(P, -1),
            idt,
        ],
        axis=1,
    ).astype(bf16)
    tab = np.ascontiguousarray(tab)

    in_maps = []
    for core in range(N_CORES):
        h0 = core * HPC
        # q[p, r, x, h, c, k], v[p, c16, h, d]
        q = np.empty((P, 2, 2, HPC, 8, HD), np.float32)
        v = np.empty((P, NT, HPC, D), np.float32)
        for h in range(HPC):
            qh = r(Qs[h0 + h]).reshape(P, 2, 8, D)  # [p, r, c, d]
            q[:, :, 0, h] = qh[:, :, :, :HD]
            q[:, :, 1, h] = qh[:, :, :, HD:]
            v[:, :, h] = r(Vs[h0 + h])
        in_maps.append(
            {
                "TAB": tab,
                "QA": np.ascontiguousarray(q[:, 0].reshape(P, -1).astype(bf16)),
                "QB": np.ascontiguousarray(q[:, 1].reshape(P, -1).astype(bf16)),
                "VA": np.ascontiguousarray(
                    v[:, 0:8].reshape(P, -1).astype(bf16)
                ),
                "VB": np.ascontiguousarray(
                    v[:, 8:16].reshape(P, -1).astype(bf16)
                ),
            }
        )
    return in_maps


def _unpack_out(o):
    # o: [P, T] = outT; rows h*64+j, cols c-major: col = u*128 + f, t = f*16+u
    a = o.reshape(HPC, D, NT, P)  # [h, j, u, f]
    return a.transpose(0, 3, 2, 1).reshape(HPC, T, D)  # [h, t=f*16+u, j]


def run_inner(Q, K, V, trace=False):
    del K  # the module sets KR = QR; K is unused
    Qs = np.asarray(Q, dtype=np.float32)[0]  # [H, T, D]
    Vs = np.asarray(V, dtype=np.float32)[0]
    cos32, sin32 = _rope_tables()
    idt = np.eye(P, dtype=np.float32)
    nc = _get_nc()
    in_maps = _pack_inputs(Qs, Vs, cos32, sin32, idt)
    res = run_bass_kernel_spmd(nc, in_maps, list(range(N_CORES)), trace=trace)
    outs = [_unpack_out(np.asarray(res.results[i]["OUT"])) for i in range(N_CORES)]
    out = np.concatenate(outs, axis=0)[None]  # [1, H, T, D]
    return out.astype(np.float32), res


def kernel(Q, K, V):
    out, _ = run_inner(Q, K, V, trace=False)
    return out


# revision 12
# speedup vs baseline: 1.2241x; 1.2241x over previous
"""Trainium2 Bass kernel for nn_LinearAttention (RoPE(Q) @ RoPE(Q)^T @ V).

Algebra: no softmax, so out = (QR @ QR^T) @ V == QR @ (QR^T @ V) with a
[d,d] (64x64) intermediate per head. Sharding: 16 heads / 8 cores = 2
heads per core, no cross-core traffic. The two heads ride the two
64-lane halves of the 128x128 PE array.

Layout: t = p*16 + r*8 + c (p = SBUF partition, r = range 0/1, c =
chunk-in-range); the host packs/unpacks with this permutation.

v2 changes vs the first working kernel (31.1us):
  * Tables shipped COMPACT ([r,c,k] cos/sin, no per-head repeat: 288KB
    instead of 544KB) and broadcast over h with stride-0 APs in the
    RoPE muls -- cuts the input stream by 20%.
  * RoPE is DVE-only. GpSimd tensor ops are gone: every DVE
    tensor_tensor needs the shared SBUF read port that GpSimd locks
    for its whole (4x slower) instruction, so DVE+GpSimd elementwise
    work serializes instead of overlapping (measured 3x slowdown).
  * 12 RoPE ops of [128,512], all reads/writes contiguous (the chunk
    strides moved into the matmul lhsT APs, which tolerate them).
  * Transposes batch 4 chunks into one PSUM bank -> 4 big [128,512]
    evacuation casts on ACT instead of 16 small copies split DVE/ACT.
  * Phase-3 uses 4 distinct PSUM banks so its matmuls stream
    back-to-back; evac casts alternate DVE/ACT; output DMAs alternate
    the two HWDGE rings.
  * Input DMA instructions are hoisted into the engine preamble block
    (before the initial all-engine barrier) -- they have no waits, and
    issuing them ~0.8us earlier starts the HBM stream during the
    barrier/branch overhead.
  * PE warm-up spam sized to bridge from the preamble to the first
    real matmul, plus a short mid-kernel bridge while DVE finishes
    RoPE-B, so HAM stays at K=8/8 for the phase-3 matmuls.
"""

from contextlib import ExitStack

import numpy as np

import concourse.bass as bass
import concourse.mybir as mybir
import concourse.tile as tile
from concourse.bass_utils import run_bass_kernel_spmd
from concourse.vector_clock import ScopedClock

H, T, D = 16, 2048, 64
N_CORES = 8
HPC = H // N_CORES  # heads per core
P = 128
NT = T // P  # 16 t-chunks per head
HD = D // 2
NTAB = 2 * 2 * 8 * HD + P  # cosA|sinA|cosB|sinB ([c,k] each) | idt
F32 = mybir.dt.float32
BF16 = mybir.dt.bfloat16
N_WARM = 22  # leading dep-free matmuls: preamble -> first real MM
N_WARM_MID = 2  # bridge the PE gap while DVE finishes RoPE-B


def _rope_tables():
    inv_freq = 1.0 / (10000.0 ** (np.arange(0, D, 2, dtype=np.float32) / D))
    t = np.arange(T, dtype=np.float32)
    freqs = np.outer(t, inv_freq).astype(np.float32)  # [T, D/2]
    return np.cos(freqs).astype(np.float32), np.sin(freqs).astype(np.float32)


class _SlimTileContext(tile.TileContext):
    """TileContext whose kernel tail uses per-engine drains + a
    sequencer-level (sem-only) barrier instead of the full EVSEM
    butterfly (~8us)."""

    def _drain_and_barrier(self, tick_clock, wait_clock):
        nc = self.nc
        drain_inst = nc.sync.drain()
        wait_clock.add_sem_waits(
            drain_inst.ins, ScopedClock({None: tick_clock.global_clock})
        )
        for eng in nc.engines.values():
            if eng.engine != mybir.EngineType.SP:
                eng.drain(fusable=False)
        nc.all_engine_barrier(sem_only=True)
        popped = nc._tile_sem_poison_stack.pop()
        assert popped is self._sem_poison
        nc.clear_and_free_semaphores(list(self.sems.allocated().values()))
        nc.all_engine_barrier(sem_only=True)


def _build_nc():
    nc = bass.Bass()
    TAB = nc.declare_dram_parameter("TAB", [P, NTAB], BF16, isOutput=False)
    # q: [p, (x h c k)] per range (x = rotate-half half, h = head)
    QA = nc.declare_dram_parameter("QA", [P, 1024], BF16, isOutput=False)
    QB = nc.declare_dram_parameter("QB", [P, 1024], BF16, isOutput=False)
    # v: [p, (c h d)] per range
    VA = nc.declare_dram_parameter("VA", [P, 1024], BF16, isOutput=False)
    VB = nc.declare_dram_parameter("VB", [P, 1024], BF16, isOutput=False)
    OUT = nc.declare_dram_parameter("OUT", [P, T], BF16, isOutput=True)

    with _SlimTileContext(nc) as tc, ExitStack() as ctx:
        singles = ctx.enter_context(tc.tile_pool(name="singles", bufs=1))
        ps_s = ctx.enter_context(tc.tile_pool(name="ps_s", bufs=1, space="PSUM"))
        ps_tp = ctx.enter_context(tc.tile_pool(name="ps_tp", bufs=2, space="PSUM"))
        ps_o = ctx.enter_context(tc.tile_pool(name="ps_o", bufs=4, space="PSUM"))
        ps_w = ctx.enter_context(tc.tile_pool(name="ps_w", bufs=1, space="PSUM"))

        tab_sb = singles.tile([P, NTAB], BF16)
        # q layout: [p, r, x, h, c, k]
        q_sb = singles.tile([P, 2, 2, HPC, 8, HD], BF16)
        v_sb = singles.tile([P, NT, P], BF16)
        # chunk-major so each chunk's (h,x,k) is a contiguous 128-elem
        # lhsT slice (matmul stationary APs allow only one free dim)
        qr_sb = singles.tile([P, NT, HPC, 2, HD], BF16)
        tm = singles.tile([P, 4, HPC, 8, HD], BF16)
        qrt_sb = singles.tile([P, NT * P], BF16)
        s2d = singles.tile([P, P], BF16)
        outT_sb = singles.tile([P, T], BF16)
        spam_src = singles.tile([P, P], F32)

        # s2d off-diagonal zeros + spam seed: GpSimd is otherwise unused
        # and runs these during the DMA wait, before any DVE op exists
        # to contend with on the shared SBUF port.
        nc.gpsimd.memset(spam_src[:, 0:2], 0.0)
        nc.gpsimd.memset(s2d[0:D, D:P], 0.0)
        nc.gpsimd.memset(s2d[D:P, 0:D], 0.0)

        # Input DMAs, split fine so the earliest consumers unblock
        # sooner. sync: QA-lo, QA-hi, QB-lo, QB-hi, VB; scalar: TAB-A,
        # TAB-B+idt, VA. (_hoist_input_dmas moves all of these into the
        # preamble block so the HBM stream starts during the barrier.)
        def qview(dram, x):
            return dram[:, x * 512 : (x + 1) * 512].rearrange(
                "p (h c k) -> p h c k", h=HPC, c=8
            )

        nc.sync.dma_start(out=q_sb[:, 0, 0], in_=qview(QA, 0))
        nc.scalar.dma_start(out=tab_sb[:, 0:512], in_=TAB[:, 0:512])
        nc.sync.dma_start(out=q_sb[:, 0, 1], in_=qview(QA, 1))
        nc.scalar.dma_start(out=tab_sb[:, 512:NTAB], in_=TAB[:, 512:NTAB])
        nc.sync.dma_start(out=q_sb[:, 1, 0], in_=qview(QB, 0))
        nc.scalar.dma_start(
            out=v_sb[:, 0:8], in_=VA[:].rearrange("p (c f) -> p c f", c=8)
        )
        nc.sync.dma_start(out=q_sb[:, 1, 1], in_=qview(QB, 1))
        nc.sync.dma_start(
            out=v_sb[:, 8:16], in_=VB[:].rearrange("p (c f) -> p c f", c=8)
        )

        # Garbage-input PE warm-up: dep-free REGULAR matmuls (transpose
        # mode may not register as PE-busy for HAM) into rotating slices
        # of one preallocated PSUM bank -- slices avoid the tile-pool
        # recycling semaphores that would serialize the PE queue.
        spam_ps = ps_w.tile([P, 512], F32)
        for i in range(N_WARM):
            j = i % 4
            nc.tensor.matmul(
                spam_ps[:, j * P : (j + 1) * P], lhsT=spam_src, rhs=spam_src,
                start=True, stop=True, skip_group_check=True,
            )

        idt = tab_sb[:, 4 * 256 :]  # [P, 128] identity

        def rope(r):
            # 6 contiguous [128,512] DVE ops; cos/sin broadcast over h.
            cosB = (
                tab_sb[:, r * 512 : r * 512 + 256]
                .rearrange("p (c k) -> p c k", c=8)
                .unsqueeze(1)
                .to_broadcast([P, HPC, 8, HD])
            )
            sinB = (
                tab_sb[:, r * 512 + 256 : r * 512 + 512]
                .rearrange("p (c k) -> p c k", c=8)
                .unsqueeze(1)
                .to_broadcast([P, HPC, 8, HD])
            )
            qlo = q_sb[:, r, 0]
            qhi = q_sb[:, r, 1]
            cs = slice(r * 8, r * 8 + 8)
            # combine dests scatter into the chunk-major qr tile using
            # the same (h, c, k) iteration order as the contiguous srcs
            qr_lo = qr_sb[:, cs, :, 0, :].rearrange("p c h k -> p h c k")
            qr_hi = qr_sb[:, cs, :, 1, :].rearrange("p c h k -> p h c k")
            # q-lo muls first: the lo-half DMA lands ~0.7us before hi
            nc.vector.tensor_mul(tm[:, 0], qlo, cosB)
            nc.vector.tensor_mul(tm[:, 3], qlo, sinB)
            nc.vector.tensor_mul(tm[:, 1], qhi, sinB)
            nc.vector.tensor_sub(qr_lo, tm[:, 0], tm[:, 1])
            nc.vector.tensor_mul(tm[:, 2], qhi, cosB)
            nc.vector.tensor_add(qr_hi, tm[:, 2], tm[:, 3])

        s2_ps = ps_s.tile([P, P], F32)

        def phase2(r):
            # per chunk: one LDW (shared) + accum MM + transpose MM;
            # transposes batch 4 chunks per PSUM bank, evacuated by ACT
            # as one [128,512] cast each.
            for ci in range(8):
                c = r * 8 + ci
                if c % 4 == 0:
                    tp = ps_tp.tile([P, 512], F32, tag="tp")
                    phase2.tp = tp
                tp = phase2.tp
                qr_c = qr_sb[:, c].rearrange("p h x k -> p (h x k)")
                nc.tensor.matmul(
                    s2_ps, lhsT=qr_c, rhs=v_sb[:, c],
                    start=(c == 0), stop=(c == NT - 1),
                )
                j = c % 4
                nc.tensor.matmul(
                    tp[:, j * P : (j + 1) * P], lhsT=qr_c, rhs=idt,
                    start=True, stop=True,
                )
                if c % 4 == 3:
                    # groups 0,1 evac while DVE still runs RoPE-B; 2,3
                    # after RoPE is done, when DVE is free
                    g = c // 4
                    dst = qrt_sb[:, g * 512 : (g + 1) * 512]
                    if g < 2:
                        nc.scalar.copy(out=dst, in_=tp)
                    else:
                        nc.vector.tensor_copy(out=dst, in_=tp)

        rope(0)
        phase2(0)
        rope(1)
        # Bridge the PE idle window while DVE finishes RoPE-B.
        for i in range(N_WARM_MID):
            j = i % 4
            nc.tensor.matmul(
                spam_ps[:, j * P : (j + 1) * P], lhsT=spam_src, rhs=spam_src,
                start=True, stop=True, skip_group_check=True,
            )
        phase2(1)

        # Diagonal S_h blocks -> block-diagonal phase-3 operand (ACT:
        # DVE is busy with the late qrt evacuations).
        nc.scalar.copy(out=s2d[0:D, 0:D], in_=s2_ps[0:D, 0:D])
        nc.scalar.copy(out=s2d[D:P, D:P], in_=s2_ps[D:P, D:P])

        # outT blocks: blockdiag(S)^T @ QRT serves both heads at once.
        # 4 distinct PSUM banks; evac casts alternate DVE/ACT; output
        # DMAs alternate the two rings.
        for i in range(4):
            o_ps = ps_o.tile([P, 512], F32, tag="o")
            blk = slice(i * 512, (i + 1) * 512)
            nc.tensor.matmul(
                o_ps, lhsT=s2d, rhs=qrt_sb[:, blk], start=True, stop=True
            )
            if i % 2 == 0:
                nc.vector.tensor_copy(out=outT_sb[:, blk], in_=o_ps)
                nc.sync.dma_start(out=OUT[:, blk], in_=outT_sb[:, blk])
            else:
                nc.scalar.copy(out=outT_sb[:, blk], in_=o_ps)
                nc.scalar.dma_start(out=OUT[:, blk], in_=outT_sb[:, blk])

    _split_multi_waits(nc)
    _hoist_input_dmas(nc)
    return nc


def _split_multi_waits(nc):
    """This compiler build rejects instructions carrying more than one
    sync-wait command: split extras into single-wait NoOps placed
    immediately before on the same engine."""
    n = 0
    for f in nc.m.functions:
        for blk in f.blocks:
            new_insts = []
            for inst in blk.instructions:
                si = inst.sync_info
                waits = list(si.on_wait) if si else []
                if len(waits) > 1:
                    for w in waits[:-1]:
                        nop = mybir.InstNoOp(name=f"W-split-{n}", ins=[], outs=[])
                        n += 1
                        nop.engine = inst.engine
                        nop.sync_info = mybir.SyncInfo(on_wait=[w], on_update=[])
                        new_insts.append(nop)
                    inst.sync_info = mybir.SyncInfo(
                        on_wait=[waits[-1]], on_update=list(si.on_update)
                    )
                new_insts.append(inst)
            blk.instructions = new_insts


def _hoist_input_dmas(nc):
    """Move the wait-free input DMA issues from the main block into the
    preamble block, before each issuing engine's drain+barrier, so the
    HBM stream starts ~0.8us earlier (during barrier/branch overhead)."""
    f = nc.m.functions[0]
    if len(f.blocks) < 2:
        return
    pre, main = f.blocks[0], f.blocks[1]
    hoist = []
    for inst in list(main.instructions):
        if isinstance(inst, mybir.InstDMACopy):
            si = inst.sync_info
            if si is not None and len(si.on_wait) > 0:
                continue
            # input loads only: DRAM source (ins reference a DRAM tensor)
            srcs = [x.memref for x in inst.ins] if inst.ins else []
            if any(n.startswith(("QA", "QB", "VA", "VB", "TAB")) for n in srcs):
                hoist.append(inst)
    if not hoist:
        return
    for inst in hoist:
        main.instructions.remove(inst)
    # insert before the issuing engine's first InstDrain in the preamble,
    # preserving issue order within each engine
    for inst in hoist:
        idx = next(
            (
                i
                for i, pi in enumerate(pre.instructions)
                if isinstance(pi, mybir.InstDrain) and pi.engine == inst.engine
            ),
            None,
        )
        if idx is None:
            main.instructions.insert(0, inst)
        else:
            pre.instructions.insert(idx, inst)


_NC_CACHE = None


def _get_nc():
    global _NC_CACHE
    if _NC_CACHE is None:
        _NC_CACHE = _build_nc()
    return _NC_CACHE


def _pack_inputs(Qs, Vs, cos32, sin32, idt):
    import ml_dtypes

    bf16 = ml_dtypes.bfloat16

    # [T, X] -> [P, NT, X] with t = p*NT + u, u = r*8 + c
    def r(x):
        return x.reshape(P, NT, -1)

    # compact tables: tab[p, r, c, k] = cos((p*16 + r*8 + c) * w_k)
    ce = r(cos32).reshape(P, 2, 8, HD)  # [p, r, c, k]
    se = r(sin32).reshape(P, 2, 8, HD)
    tab = np.concatenate(
        [
            ce[:, 0].reshape(P, -1),
            se[:, 0].reshape(P, -1),
            ce[:, 1].reshape(P, -1),
            se[:, 1].reshape(P, -1),
            idt,
        ],
        axis=1,
    ).astype(bf16)
    tab = np.ascontiguousarray(tab)

    in_maps = []
    for core in range(N_CORES):
        h0 = core * HPC
        # q[p, r, x, h, c, k], v[p, c16, h, d]
        q = np.empty((P, 2, 2, HPC, 8, HD), np.float32)
        v = np.empty((P, NT, HPC, D), np.float32)
        for h in range(HPC):
            qh = r(Qs[h0 + h]).reshape(P, 2, 8, D)  # [p, r, c, d]
            q[:, :, 0, h] = qh[:, :, :, :HD]
            q[:, :, 1, h] = qh[:, :, :, HD:]
            v[:, :, h] = r(Vs[h0 + h])
        in_maps.append(
            {
                "TAB": tab,
                "QA": np.ascontiguousarray(q[:, 0].reshape(P, -1).astype(bf16)),
                "QB": np.ascontiguousarray(q[:, 1].reshape(P, -1).astype(bf16)),
                "VA": np.ascontiguousarray(
                    v[:, 0:8].reshape(P, -1).astype(bf16)
                ),
                "VB": np.ascontiguousarray(
                    v[:, 8:16].reshape(P, -1).astype(bf16)
                ),
            }
        )
    return in_maps


def _unpack_out(o):
    # o: [P, T] = outT; rows h*64+j, cols c-major: col = u*128 + f, t = f*16+u
    a = o.reshape(HPC, D, NT, P)  # [h, j, u, f]
    return a.transpose(0, 3, 2, 1).reshape(HPC, T, D)  # [h, t=f*16+u, j]


def run_inner(Q, K, V, trace=False):
    del K  # the module sets KR = QR; K is unused
    Qs = np.asarray(Q, dtype=np.float32)[0]  # [H, T, D]
    Vs = np.asarray(V, dtype=np.float32)[0]
    cos32, sin32 = _rope_tables()
    idt = np.eye(P, dtype=np.float32)
    nc = _get_nc()
    in_maps = _pack_inputs(Qs, Vs, cos32, sin32, idt)
    res = run_bass_kernel_spmd(nc, in_maps, list(range(N_CORES)), trace=trace)
    outs = [_unpack_out(np.asarray(res.results[i]["OUT"])) for i in range(N_CORES)]
    out = np.concatenate(outs, axis=0)[None]  # [1, H, T, D]
    return out.astype(np.float32), res


def kernel(Q, K, V):
    out, _ = run_inner(Q, K, V, trace=False)
    return out


# revision 16
# speedup vs baseline: 1.3126x; 1.0723x over previous
"""Trainium2 Bass kernel for nn_LinearAttention (RoPE(Q) @ RoPE(Q)^T @ V).

Algebra: no softmax, so out = (QR @ QR^T) @ V == QR @ (QR^T @ V) with a
[d,d] (64x64) intermediate per head. Sharding: 16 heads / 8 cores = 2
heads per core, no cross-core traffic. The two heads ride the two
64-lane halves of the 128x128 PE array.

Layout: t = p*16 + r*8 + c (p = SBUF partition, r = range 0/1, c =
chunk-in-range); the host packs/unpacks with this permutation.

v2 changes vs the first working kernel (31.1us):
  * Tables shipped COMPACT ([r,c,k] cos/sin, no per-head repeat: 288KB
    instead of 544KB) and broadcast over h with stride-0 APs in the
    RoPE muls -- cuts the input stream by 20%.
  * RoPE is DVE-only. GpSimd tensor ops are gone: every DVE
    tensor_tensor needs the shared SBUF read port that GpSimd locks
    for its whole (4x slower) instruction, so DVE+GpSimd elementwise
    work serializes instead of overlapping (measured 3x slowdown).
  * 12 RoPE ops of [128,512], all reads/writes contiguous (the chunk
    strides moved into the matmul lhsT APs, which tolerate them).
  * Transposes batch 4 chunks into one PSUM bank -> 4 big [128,512]
    evacuation casts on ACT instead of 16 small copies split DVE/ACT.
  * Phase-3 uses 4 distinct PSUM banks so its matmuls stream
    back-to-back; evac casts alternate DVE/ACT; output DMAs alternate
    the two HWDGE rings.
  * Input DMA instructions are hoisted into the engine preamble block
    (before the initial all-engine barrier) -- they have no waits, and
    issuing them ~0.8us earlier starts the HBM stream during the
    barrier/branch overhead.
  * PE warm-up spam sized to bridge from the preamble to the first
    real matmul, plus a short mid-kernel bridge while DVE finishes
    RoPE-B, so HAM stays at K=8/8 for the phase-3 matmuls.
"""

from contextlib import ExitStack

import numpy as np

import concourse.bass as bass
import concourse.mybir as mybir
import concourse.tile as tile
from concourse.bass_utils import run_bass_kernel_spmd
from concourse.vector_clock import ScopedClock

H, T, D = 16, 2048, 64
N_CORES = 8
HPC = H // N_CORES  # heads per core
P = 128
NT = T // P  # 16 t-chunks per head
HD = D // 2
NTAB = 2 * 2 * 8 * HD + P  # cosA|sinA|cosB|sinB ([c,k] each) | idt
F32 = mybir.dt.float32
BF16 = mybir.dt.bfloat16
N_WARM = 19  # leading dep-free matmuls: preamble -> first real MM
N_WARM_MID = 4  # bridge the PE gap while DVE finishes RoPE-B


def _rope_tables():
    inv_freq = 1.0 / (10000.0 ** (np.arange(0, D, 2, dtype=np.float32) / D))
    t = np.arange(T, dtype=np.float32)
    freqs = np.outer(t, inv_freq).astype(np.float32)  # [T, D/2]
    return np.cos(freqs).astype(np.float32), np.sin(freqs).astype(np.float32)


class _SlimTileContext(tile.TileContext):
    """TileContext whose kernel tail uses per-engine drains + a
    sequencer-level (sem-only) barrier instead of the full EVSEM
    butterfly (~8us)."""

    def _drain_and_barrier(self, tick_clock, wait_clock):
        nc = self.nc
        drain_inst = nc.sync.drain()
        wait_clock.add_sem_waits(
            drain_inst.ins, ScopedClock({None: tick_clock.global_clock})
        )
        for eng in nc.engines.values():
            if eng.engine != mybir.EngineType.SP:
                eng.drain(fusable=False)
        nc.all_engine_barrier(sem_only=True)
        popped = nc._tile_sem_poison_stack.pop()
        assert popped is self._sem_poison
        nc.clear_and_free_semaphores(list(self.sems.allocated().values()))
        nc.all_engine_barrier(sem_only=True)


def _build_nc():
    nc = bass.Bass()
    TAB = nc.declare_dram_parameter("TAB", [P, NTAB], BF16, isOutput=False)
    # q: [p, (x h c k)] per range (x = rotate-half half, h = head)
    QA = nc.declare_dram_parameter("QA", [P, 1024], BF16, isOutput=False)
    QB = nc.declare_dram_parameter("QB", [P, 1024], BF16, isOutput=False)
    # v: [p, (c h d)] per range
    VA = nc.declare_dram_parameter("VA", [P, 1024], BF16, isOutput=False)
    VB = nc.declare_dram_parameter("VB", [P, 1024], BF16, isOutput=False)
    OUT = nc.declare_dram_parameter("OUT", [P, T], BF16, isOutput=True)

    with _SlimTileContext(nc) as tc, ExitStack() as ctx:
        singles = ctx.enter_context(tc.tile_pool(name="singles", bufs=1))
        ps_s = ctx.enter_context(tc.tile_pool(name="ps_s", bufs=1, space="PSUM"))
        ps_tp = ctx.enter_context(tc.tile_pool(name="ps_tp", bufs=2, space="PSUM"))
        ps_o = ctx.enter_context(tc.tile_pool(name="ps_o", bufs=4, space="PSUM"))
        ps_w = ctx.enter_context(tc.tile_pool(name="ps_w", bufs=1, space="PSUM"))

        tab_sb = singles.tile([P, NTAB], BF16)
        # q layout: [p, r, x, h, c, k]
        q_sb = singles.tile([P, 2, 2, HPC, 8, HD], BF16)
        v_sb = singles.tile([P, NT, P], BF16)
        # chunk-major so each chunk's (h,x,k) is a contiguous 128-elem
        # lhsT slice (matmul stationary APs allow only one free dim)
        qr_sb = singles.tile([P, NT, HPC, 2, HD], BF16)
        tm = singles.tile([P, 4, HPC, 8, HD], BF16)
        qrt_sb = singles.tile([P, NT * P], BF16)
        s2d = singles.tile([P, P], BF16)
        outT_sb = singles.tile([P, T], BF16)
        spam_src = singles.tile([P, P], F32)

        # s2d off-diagonal zeros + spam seed: GpSimd is otherwise unused
        # and runs these during the DMA wait, before any DVE op exists
        # to contend with on the shared SBUF port.
        nc.gpsimd.memset(spam_src[:, 0:2], 0.0)
        nc.gpsimd.memset(s2d[0:D, D:P], 0.0)
        nc.gpsimd.memset(s2d[D:P, 0:D], 0.0)

        # Input DMAs, split fine so the earliest consumers unblock
        # sooner. sync: QA-lo, QA-hi, QB-lo, QB-hi, VB; scalar: TAB-A,
        # TAB-B+idt, VA. (_hoist_input_dmas moves all of these into the
        # preamble block so the HBM stream starts during the barrier.)
        def qview(dram, x):
            return dram[:, x * 512 : (x + 1) * 512].rearrange(
                "p (h c k) -> p h c k", h=HPC, c=8
            )

        nc.sync.dma_start(out=q_sb[:, 0, 0], in_=qview(QA, 0))
        nc.scalar.dma_start(out=tab_sb[:, 0:512], in_=TAB[:, 0:512])
        nc.sync.dma_start(out=q_sb[:, 0, 1], in_=qview(QA, 1))
        nc.scalar.dma_start(out=tab_sb[:, 512:NTAB], in_=TAB[:, 512:NTAB])
        nc.sync.dma_start(
            out=q_sb[:, 1],
            in_=QB[:].rearrange("p (x h c k) -> p x h c k", x=2, h=HPC, c=8),
        )
        nc.scalar.dma_start(
            out=v_sb[:, 0:8], in_=VA[:].rearrange("p (c f) -> p c f", c=8)
        )
        nc.sync.dma_start(
            out=v_sb[:, 8:16], in_=VB[:].rearrange("p (c f) -> p c f", c=8)
        )

        # Garbage-input PE warm-up: dep-free REGULAR matmuls (transpose
        # mode may not register as PE-busy for HAM) into rotating slices
        # of one preallocated PSUM bank -- slices avoid the tile-pool
        # recycling semaphores that would serialize the PE queue.
        spam_ps = ps_w.tile([P, 512], F32)
        for i in range(N_WARM):
            j = i % 4
            nc.tensor.matmul(
                spam_ps[:, j * P : (j + 1) * P], lhsT=spam_src, rhs=spam_src,
                start=True, stop=True, skip_group_check=True,
            )

        idt = tab_sb[:, 4 * 256 :]  # [P, 128] identity

        def rope(r):
            # 6 contiguous [128,512] DVE ops; cos/sin broadcast over h.
            cosB = (
                tab_sb[:, r * 512 : r * 512 + 256]
                .rearrange("p (c k) -> p c k", c=8)
                .unsqueeze(1)
                .to_broadcast([P, HPC, 8, HD])
            )
            sinB = (
                tab_sb[:, r * 512 + 256 : r * 512 + 512]
                .rearrange("p (c k) -> p c k", c=8)
                .unsqueeze(1)
                .to_broadcast([P, HPC, 8, HD])
            )
            qlo = q_sb[:, r, 0]
            qhi = q_sb[:, r, 1]
            cs = slice(r * 8, r * 8 + 8)
            # combine dests scatter into the chunk-major qr tile using
            # the same (h, c, k) iteration order as the contiguous srcs
            qr_lo = qr_sb[:, cs, :, 0, :].rearrange("p c h k -> p h c k")
            qr_hi = qr_sb[:, cs, :, 1, :].rearrange("p c h k -> p h c k")
            # q-lo muls first: the lo-half DMA lands ~0.7us before hi
            nc.vector.tensor_mul(tm[:, 0], qlo, cosB)
            nc.vector.tensor_mul(tm[:, 3], qlo, sinB)
            nc.vector.tensor_mul(tm[:, 1], qhi, sinB)
            nc.vector.tensor_sub(qr_lo, tm[:, 0], tm[:, 1])
            nc.vector.tensor_mul(tm[:, 2], qhi, cosB)
            nc.vector.tensor_add(qr_hi, tm[:, 2], tm[:, 3])

        s2_ps = ps_s.tile([P, P], F32)

        def phase2(r):
            # per chunk: one LDW (shared) + accum MM + transpose MM;
            # transposes batch 4 chunks per PSUM bank, evacuated by ACT
            # as one [128,512] cast each.
            for ci in range(8):
                c = r * 8 + ci
                if c % 4 == 0:
                    tp = ps_tp.tile([P, 512], F32, tag="tp")
                    phase2.tp = tp
                tp = phase2.tp
                qr_c = qr_sb[:, c].rearrange("p h x k -> p (h x k)")
                nc.tensor.matmul(
                    s2_ps, lhsT=qr_c, rhs=v_sb[:, c],
                    start=(c == 0), stop=(c == NT - 1),
                )
                j = c % 4
                nc.tensor.matmul(
                    tp[:, j * P : (j + 1) * P], lhsT=qr_c, rhs=idt,
                    start=True, stop=True,
                )
                if c % 4 == 3:
                    # ACT takes groups 0-2 (DVE busy with RoPE); DVE
                    # (free after RoPE-B) takes the last group
                    g = c // 4
                    dst = qrt_sb[:, g * 512 : (g + 1) * 512]
                    if g < 3:
                        nc.scalar.copy(out=dst, in_=tp)
                    else:
                        nc.vector.tensor_copy(out=dst, in_=tp)

        rope(0)
        phase2(0)
        rope(1)
        # Bridge the PE idle window while DVE finishes RoPE-B.
        for i in range(N_WARM_MID):
            j = i % 4
            nc.tensor.matmul(
                spam_ps[:, j * P : (j + 1) * P], lhsT=spam_src, rhs=spam_src,
                start=True, stop=True, skip_group_check=True,
            )
        phase2(1)

        # Diagonal S_h blocks -> block-diagonal phase-3 operand (ACT:
        # DVE is busy with the late qrt evacuations).
        nc.scalar.copy(out=s2d[0:D, 0:D], in_=s2_ps[0:D, 0:D])
        nc.scalar.copy(out=s2d[D:P, D:P], in_=s2_ps[D:P, D:P])

        # outT blocks: blockdiag(S)^T @ QRT serves both heads at once.
        # 4 distinct PSUM banks; evac casts alternate DVE/ACT; output
        # DMAs alternate the two rings.
        for i in range(4):
            o_ps = ps_o.tile([P, 512], F32, tag="o")
            blk = slice(i * 512, (i + 1) * 512)
            nc.tensor.matmul(
                o_ps, lhsT=s2d, rhs=qrt_sb[:, blk], start=True, stop=True
            )
            if i % 2 == 0:
                nc.vector.tensor_copy(out=outT_sb[:, blk], in_=o_ps)
                nc.sync.dma_start(out=OUT[:, blk], in_=outT_sb[:, blk])
            else:
                nc.scalar.copy(out=outT_sb[:, blk], in_=o_ps)
                nc.scalar.dma_start(out=OUT[:, blk], in_=outT_sb[:, blk])

    _split_multi_waits(nc)
    _hoist_input_dmas(nc)
    return nc


def _split_multi_waits(nc):
    """This compiler build rejects instructions carrying more than one
    sync-wait command: split extras into single-wait NoOps placed
    immediately before on the same engine."""
    n = 0
    for f in nc.m.functions:
        for blk in f.blocks:
            new_insts = []
            for inst in blk.instructions:
                si = inst.sync_info
                waits = list(si.on_wait) if si else []
                if len(waits) > 1:
                    for w in waits[:-1]:
                        nop = mybir.InstNoOp(name=f"W-split-{n}", ins=[], outs=[])
                        n += 1
                        nop.engine = inst.engine
                        nop.sync_info = mybir.SyncInfo(on_wait=[w], on_update=[])
                        new_insts.append(nop)
                    inst.sync_info = mybir.SyncInfo(
                        on_wait=[waits[-1]], on_update=list(si.on_update)
                    )
                new_insts.append(inst)
            blk.instructions = new_insts


def _hoist_input_dmas(nc):
    """Move the wait-free input DMA issues from the main block into the
    preamble block, right AFTER each issuing engine's barrier-increment
    Drain (a DMA issue costs ~0.65us on the issuing engine; placing it
    before the Drain would hold the all-engine barrier and stall every
    other engine behind the whole DMA issue train)."""
    f = nc.m.functions[0]
    if len(f.blocks) < 2:
        return
    pre, main = f.blocks[0], f.blocks[1]
    hoist = []
    for inst in list(main.instructions):
        if isinstance(inst, mybir.InstDMACopy):
            si = inst.sync_info
            if si is not None and len(si.on_wait) > 0:
                continue
            # input loads only: DRAM source (ins reference a DRAM tensor)
            srcs = [x.memref for x in inst.ins] if inst.ins else []
            if any(n.startswith(("QA", "QB", "VA", "VB", "TAB")) for n in srcs):
                hoist.append(inst)
    if not hoist:
        return
    for inst in hoist:
        main.instructions.remove(inst)
    # insert right after the issuing engine's first InstDrain (which
    # carries the barrier increment), preserving issue order per engine
    for inst in reversed(hoist):
        idx = next(
            (
                i
                for i, pi in enumerate(pre.instructions)
                if isinstance(pi, mybir.InstDrain) and pi.engine == inst.engine
            ),
            None,
        )
        if idx is None:
            main.instructions.insert(0, inst)
        else:
            pre.instructions.insert(idx + 1, inst)


_NC_CACHE = None


def _get_nc():
    global _NC_CACHE
    if _NC_CACHE is None:
        _NC_CACHE = _build_nc()
    return _NC_CACHE


def _pack_inputs(Qs, Vs, cos32, sin32, idt):
    import ml_dtypes

    bf16 = ml_dtypes.bfloat16

    # [T, X] -> [P, NT, X] with t = p*NT + u, u = r*8 + c
    def r(x):
        return x.reshape(P, NT, -1)

    # compact tables: tab[p, r, c, k] = cos((p*16 + r*8 + c) * w_k)
    ce = r(cos32).reshape(P, 2, 8, HD)  # [p, r, c, k]
    se = r(sin32).reshape(P, 2, 8, HD)
    tab = np.concatenate(
        [
            ce[:, 0].reshape(P, -1),
            se[:, 0].reshape(P, -1),
            ce[:, 1].reshape(P, -1),
            se[:, 1].reshape(P, -1),
            idt,
        ],
        axis=1,
    ).astype(bf16)
    tab = np.ascontiguousarray(tab)

    in_maps = []
    for core in range(N_CORES):
        h0 = core * HPC
        # q[p, r, x, h, c, k], v[p, c16, h, d]
        q = np.empty((P, 2, 2, HPC, 8, HD), np.float32)
        v = np.empty((P, NT, HPC, D), np.float32)
        for h in range(HPC):
            qh = r(Qs[h0 + h]).reshape(P, 2, 8, D)  # [p, r, c, d]
            q[:, :, 0, h] = qh[:, :, :, :HD]
            q[:, :, 1, h] = qh[:, :, :, HD:]
            v[:, :, h] = r(Vs[h0 + h])
        in_maps.append(
            {
                "TAB": tab,
                "QA": np.ascontiguousarray(q[:, 0].reshape(P, -1).astype(bf16)),
                "QB": np.ascontiguousarray(q[:, 1].reshape(P, -1).astype(bf16)),
                "VA": np.ascontiguousarray(
                    v[:, 0:8].reshape(P, -1).astype(bf16)
                ),
                "VB": np.ascontiguousarray(
                    v[:, 8:16].reshape(P, -1).astype(bf16)
                ),
            }
        )
    return in_maps


def _unpack_out(o):
    # o: [P, T] = outT; rows h*64+j, cols c-major: col = u*128 + f, t = f*16+u
    a = o.reshape(HPC, D, NT, P)  # [h, j, u, f]
    return a.transpose(0, 3, 2, 1).reshape(HPC, T, D)  # [h, t=f*16+u, j]


def run_inner(Q, K, V, trace=False):
    del K  # the module sets KR = QR; K is unused
    Qs = np.asarray(Q, dtype=np.float32)[0]  # [H, T, D]
    Vs = np.asarray(V, dtype=np.float32)[0]
    cos32, sin32 = _rope_tables()
    idt = np.eye(P, dtype=np.float32)
    nc = _get_nc()
    in_maps = _pack_inputs(Qs, Vs, cos32, sin32, idt)
    res = run_bass_kernel_spmd(nc, in_maps, list(range(N_CORES)), trace=trace)
    outs = [_unpack_out(np.asarray(res.results[i]["OUT"])) for i in range(N_CORES)]
    out = np.concatenate(outs, axis=0)[None]  # [1, H, T, D]
    return out.astype(np.float32), res


def kernel(Q, K, V):
    out, _ = run_inner(Q, K, V, trace=False)
    return out
